# revision 37
# baseline (speedup 1.0000x reference)
"""MoE FFN (top-2 of 8 experts) Trainium2 kernel.

Strategy (expert-parallel across 8 NeuronCores):
  - Host computes the (tiny) router: logits = x@Wg, softmax, top-2,
    renormalized combine weights.  Tokens are gathered per expert on the
    host ("all-to-all dispatch" done at sharding time), transposed to
    [H, C] so both FFN GEMMs run with natural weight layouts on device.
  - Core e runs the FFN for expert e over its C_pad gathered tokens,
    F-block by F-block (block weights stream through SBUF in half-block
    ring buffers; chunks of <=512 tokens bound PSUM usage):
        hT = gelu_tanh(W1.T-tiles @ xT)        # [Fb, C] per block
        Y_fb = hT-tiles.T @ W2_fb              # [C, H] partial per block
    Partials land in per-block DRAM regions as bf16; the host sums them
    in fp32 and applies combine weights + b2 ("combine").

  All matmuls are float32r (full-rate fp32).  Measured structure facts
  this kernel is built around (see trace notes):
    - warm PE does one N=512 matmul per ~227 ns; LDWEIGHTS (one per
      matmul, emitted unconditionally by bass) is hidden by the PE's
      64-deep reorder window.  bf16 matmuls measured SLOWER (259 ns).
    - The PE queue is strict FIFO and a single HWDGE ring delivers only
      ~200 GB/s -> the first F-block is small (4 f-tiles) so GEMM2 isn't
      parked behind GEMM1 waiting for W2 at startup; the last F-block is
      small too so the tail drains quickly.
    - HAM re-throttles the PE to 1.2 GHz after ~3.4 us of idle -> a few
      warm-up matmuls on a scratch tile run while the first DMAs land.
"""

import os
import sys
import numpy as np

for _p in ("/opt/trn_rl_repo", "/root/.axon_site/_ro/trn_rl_repo"):
    if _p not in sys.path and os.path.isdir(_p):
        sys.path.append(_p)

import concourse.bacc as bacc  # noqa: E402
import concourse.tile as tile  # noqa: E402
from concourse import mybir  # noqa: E402
from concourse.bass_utils import run_bass_kernel_spmd  # noqa: E402

# Problem shapes (hardcoded per spec)
B, S, H, F, E = 4, 2048, 1024, 4096, 8
T = B * S
TOP_K = 2
N_CORES = 8
P = 128
KH = H // P          # 8  H-contraction subtiles
FT = F // P          # 32 f-tiles total
BLOCKS = (4, 8, 8, 8, 4)
NBLK = len(BLOCKS)

F32 = mybir.dt.float32
F32R = mybir.dt.float32r
BF16 = mybir.dt.bfloat16

_CACHE: dict = {}
LAST_RESULT = None  # BassKernelResults of the most recent run (for test.py)


def _chunks_for(c_pad: int) -> tuple:
    """Token chunks: 512s with an optional single 256 tail."""
    out = [512] * (c_pad // 512)
    if c_pad % 512:
        assert c_pad % 512 == 256
        out.append(256)
    return tuple(out)



def _build(c_pad: int, chunks: tuple, use_b1: bool, mm_dt, act_fn=None):
    nc = bacc.Bacc(
        "TRN2",
        target_bir_lowering=False,
        debug=False,
        enable_asserts=False,
        num_devices=N_CORES,
    )

    xd = nc.dram_tensor("xd", [P, KH, c_pad], mm_dt, kind="ExternalInput").ap()
    # bf16 copy of x for blocks >= 1: re-reads go through the gpsimd
    # SWDGE cast-DMA (bf16 -> fp32), halving their HBM traffic and
    # keeping the sync ring free for block-0 x + y-out.
    xdh = nc.dram_tensor("xdh", [P, KH, c_pad], BF16, kind="ExternalInput").ap()
    w1d = nc.dram_tensor("w1d", [P, FT, KH, P], mm_dt, kind="ExternalInput").ap()
    w2d = nc.dram_tensor("w2d", [P, FT, H], mm_dt, kind="ExternalInput").ap()
    if use_b1:
        b1d = nc.dram_tensor("b1d", [P, FT], F32, kind="ExternalInput").ap()
    # per-F-block partial outputs (bf16); host sums over the NBLK axis
    yd = nc.dram_tensor(
        "yd", [P, NBLK, c_pad // P, H], BF16, kind="ExternalOutput"
    ).ap()

    gelu = act_fn or mybir.ActivationFunctionType.Gelu_apprx_tanh

    coffs = [sum(chunks[:i]) for i in range(len(chunks))]

    with tile.TileContext(nc) as tc:
        with (
            tc.tile_pool(name="w1p", bufs=2) as w1p,
            tc.tile_pool(name="w2p", bufs=2) as w2p,
            tc.tile_pool(name="xp", bufs=2) as xp,
            tc.tile_pool(name="hp", bufs=2) as hp,
            tc.tile_pool(name="op", bufs=8) as op,
            tc.tile_pool(name="bp", bufs=1) as bp,
            tc.tile_pool(name="ps1", bufs=3, space="PSUM") as ps1,
            tc.tile_pool(name="ps2", bufs=5, space="PSUM") as ps2,
        ):
            if use_b1:
                b1t = bp.tile([P, FT], F32)
                nc.sync.dma_start(b1t[:], b1d[:])

            # Pre-heat (see module docstring).  ~32 matmuls fill the
            # otherwise-dead window while the scalar ring delivers the
            # lead block's weights (~4 MB at ~200 GB/s), so real work
            # starts on a warm (2.4 GHz) PE with no HAM cold phase.
            scr = bp.tile([P, 512], BF16, name="scr")
            nc.vector.memset(scr[:], 0.0)
            for _ in range(32):
                wt = ps1.tile([P, 512], F32, tag="pt1")
                nc.tensor.matmul(
                    wt[:], scr[:, :P], scr[:], start=True, stop=True
                )

            fstart = 0
            for bi, fbn in enumerate(BLOCKS):
                half = fbn // 2
                # Full-block weight tiles, streamed on the scalar HWDGE
                # ring in PE consumption order (w1 halves then w2 halves).
                w1q = w1p.tile([P, fbn, KH, P], mm_dt, tag="w1q", name=f"w1q_{bi}")
                nc.scalar.dma_start(w1q[:, :half], w1d[:, fstart : fstart + half])
                nc.scalar.dma_start(
                    w1q[:, half:], w1d[:, fstart + half : fstart + fbn]
                )
                w2q = w2p.tile([P, fbn, H], mm_dt, tag="w2q", name=f"w2q_{bi}")
                nc.scalar.dma_start(w2q[:, :half], w2d[:, fstart : fstart + half])
                nc.scalar.dma_start(
                    w2q[:, half:], w2d[:, fstart + half : fstart + fbn]
                )

                for ci, nt in enumerate(chunks):
                    co = coffs[ci]
                    xt = xp.tile([P, KH, nt], mm_dt, tag="xt")
                    if bi == 0 and ci == 0:
                        # halves: GEMM1 k=0..3 starts on the first piece
                        nc.sync.dma_start(xt[:, :4], xd[:, :4, co : co + nt])
                        nc.sync.dma_start(xt[:, 4:], xd[:, 4:, co : co + nt])
                    elif bi == 0:
                        nc.sync.dma_start(xt[:], xd[:, :, co : co + nt])
                    else:
                        # later blocks' x rides the gpsimd cast ring (bf16 ->
                        # fp32): its ~20us startup DRAIN barrier has passed
                        # by the time block 1 needs data, and this keeps the
                        # sync ring free for block-0 x and y-out
                        nc.gpsimd.dma_start(xt[:], xdh[:, :, co : co + nt])

                    # GEMM1: hT[f, :] = gelu(sum_k W1[k, f-tile].T @ xT[k, :])
                    hq = hp.tile([P, fbn, nt], mm_dt, tag="hq", name=f"hq_{bi}")
                    for f in range(fbn):
                        pt1 = ps1.tile([P, nt], F32, tag="pt1")
                        for k in range(KH):
                            nc.tensor.matmul(
                                pt1[:],
                                w1q[:, f, k, :],
                                xt[:, k, :],
                                start=(k == 0),
                                stop=(k == KH - 1),
                            )
                        bias = (
                            b1t[:, fstart + f : fstart + f + 1] if use_b1 else 0.0
                        )
                        nc.scalar.activation(hq[:, f, :], pt1[:], gelu, bias=bias)

                    # GEMM2 (partial over this F-block):
                    # Y[t-tile, hh] += sum_k2 hT[k2, t-tile].T @ W2[k2, hh]
                    for t in range(nt // P):
                        pts = [
                            ps2.tile([P, 512], F32, tag="pt2", name=f"pt2_{hh}")
                            for hh in range(2)
                        ]
                        for k2 in range(fbn):
                            for hh in range(2):
                                nc.tensor.matmul(
                                    pts[hh][:],
                                    hq[:, k2, t * P : (t + 1) * P],
                                    w2q[:, k2, hh * 512 : (hh + 1) * 512],
                                    start=(k2 == 0),
                                    stop=(k2 == fbn - 1),
                                )
                        trow = co // P + t
                        for hh in range(2):
                            ot = op.tile([P, 512], BF16, tag="ot")
                            dst = yd[:, bi, trow, hh * 512 : (hh + 1) * 512]
                            # split the psum->bf16 evacuations across both
                            # engines so the final flush chain is half as
                            # long (scalar is the less-loaded of the two)
                            if hh == 0:
                                nc.vector.tensor_copy(ot[:], pts[hh][:])
                            else:
                                nc.scalar.activation(
                                    ot[:],
                                    pts[hh][:],
                                    mybir.ActivationFunctionType.Copy,
                                )
                            nc.sync.dma_start(dst, ot[:])
                fstart += fbn

    nc.compile()
    return nc


def _route(x2d, Wg):
    """Replicates reference router: softmax -> top-2 -> renormalize."""
    logits = x2d @ Wg  # [T, E] fp32
    m = logits.max(axis=-1, keepdims=True)
    p = np.exp(logits - m, dtype=np.float32)
    p /= p.sum(axis=-1, keepdims=True)
    # jax.lax.top_k: values descending, ties broken by lower index.
    order = np.argsort(-p, axis=-1, kind="stable")
    top_i = order[:, :TOP_K]  # [T, 2]
    top_p = np.take_along_axis(p, top_i, axis=-1)
    top_p = top_p / top_p.sum(axis=-1, keepdims=True)
    return top_i, top_p


def kernel(x, Wg, W1, b1, W2, b2):
    global LAST_RESULT
    x = np.ascontiguousarray(np.asarray(x, dtype=np.float32))
    Wg = np.ascontiguousarray(np.asarray(Wg, dtype=np.float32))
    W1 = np.ascontiguousarray(np.asarray(W1, dtype=np.float32))
    b1 = np.ascontiguousarray(np.asarray(b1, dtype=np.float32))
    W2 = np.ascontiguousarray(np.asarray(W2, dtype=np.float32))
    b2 = np.ascontiguousarray(np.asarray(b2, dtype=np.float32))

    x2d = x.reshape(T, H)
    top_i, top_p = _route(x2d, Wg)

    rows = [None] * E
    gval = [None] * E
    for e in range(E):
        r, slot = np.nonzero(top_i == e)
        rows[e] = r
        gval[e] = top_p[r, slot]

    c_max = max(len(r) for r in rows)
    c_pad = max(512, ((c_max + 255) // 256) * 256)
    chunks = _chunks_for(c_pad)
    use_b1 = bool(np.any(b1))

    key = (c_pad, chunks, use_b1)
    if key not in _CACHE:
        _CACHE[key] = _build(c_pad, chunks, use_b1, F32R)
    nc = _CACHE[key]

    in_maps = []
    for e in range(E):
        ce = len(rows[e])
        xt = np.zeros((H, c_pad), np.float32)
        xt[:, :ce] = x2d[rows[e]].T
        xre = np.ascontiguousarray(xt.reshape(KH, P, c_pad).transpose(1, 0, 2))
        m = {
            "xd": xre,
            "xdh": np.ascontiguousarray(xre.astype(mybir.dt.np(BF16))),
            "w1d": np.ascontiguousarray(
                W1[e].reshape(KH, P, FT, P).transpose(1, 2, 0, 3)
            ),
            "w2d": np.ascontiguousarray(
                W2[e].reshape(FT, P, H).transpose(1, 0, 2)
            ),
        }
        if use_b1:
            m["b1d"] = np.ascontiguousarray(b1[e].reshape(FT, P).T)
        in_maps.append(m)

    trace = os.environ.get("KERNEL_TRACE", "") == "1"
    res = run_bass_kernel_spmd(
        nc,
        in_maps,
        core_ids=list(range(N_CORES)),
        trace=trace,
        trace_cores=[0] if trace else None,
    )
    LAST_RESULT = res

    out = np.zeros((T, H), np.float32)
    for e in range(E):
        ce = len(rows[e])
        yt = res.results[e]["yd"].astype(np.float32).sum(axis=1)  # [P, c_pad//P, H]
        y = yt.transpose(1, 0, 2).reshape(c_pad, H)[:ce]
        out[rows[e]] += gval[e][:, None] * (y + b2[e][None, :])

    return out.reshape(B, S, H)
